# revision 35
# baseline (speedup 1.0000x reference)
"""Trainium2 Bass kernel for nn_BitKHopSampler.

Computes out[b, s, v] = y[b, v] + (1 - 2*y[b, v]) * mag[b, s, v] where
mag[b, s, v] = 1 iff v appears in idx[b, s, :].  Equivalently: broadcast
y[b, :] over samples, then flip each hit position v -> 1 - y[b, v].

Sharding: pure data parallel over the batch dim, 64 batches per core on
8 cores.  Per core (all shapes hardcoded):
  - y      (64, 1024)  fp32
  - idx16  (128, 256)  int16, layout [s, b*4+h], duplicate hops -> -1
  - out    (64*128, 1024) fp32

Device pipeline per batch b:
  PE    : ybc  = broadcast of y[b] to 128 partitions via K=4 bf16 matmul
          (y pre-split into exact bf16 hi/mid/lo + a constant-ones row;
          lhsT [1,1,1,0] reconstructs y exactly)
  GPSIMD: local_scatter builds the int16 hit mask from idx16
  DVE   : ybc -= mask in PSUM (y -> y or y-1 at hits)
  ACT   : Abs PSUM -> SBUF out tile (|y - mask| = y or 1-y, exact)
  DMA   : out tile -> DRAM

The kernel is one saturated HBM write stream (~33.5 MiB/core at ~400
GB/s across 16 DMA engines): exec time = stream-start time + bytes/BW +
fixed NEFF teardown.  Startup is minimized by loading batch 0's inputs
first (small split DMAs) and ramping output DMA granularity (column
halves -> single batches -> batch pairs).
"""

import numpy as np

import concourse.bacc as bacc
import concourse.bass as bass
import concourse.tile as tile
from concourse import mybir
from concourse.bass_utils import run_bass_kernel_spmd


B, S, V, H = 512, 128, 1024, 4
NCORES = 8
BL = B // NCORES  # 64 batches per core

_nc_cache = None


def _build_bass():
    nc = bacc.Bacc("TRN2", debug=False, enable_asserts=False, num_devices=NCORES)
    yspl_d = nc.dram_tensor(
        "yspl", [4 * BL, V], mybir.dt.bfloat16, kind="ExternalInput"
    ).ap()
    idx_d = nc.dram_tensor(
        "idx16", [S, BL * H], mybir.dt.int16, kind="ExternalInput"
    ).ap()
    lall_d = nc.dram_tensor(
        "lall", [96, 8 * 128], mybir.dt.bfloat16, kind="ExternalInput"
    ).ap()
    head_d = nc.dram_tensor(
        "head", [32, 2 * V], mybir.dt.bfloat16, kind="ExternalInput"
    ).ap()
    out_d = nc.dram_tensor(
        "out", [BL * S, V], mybir.dt.float32, kind="ExternalOutput"
    ).ap()

    f32 = mybir.dt.float32
    bf16 = mybir.dt.bfloat16
    Op = mybir.AluOpType

    with tile.TileContext(nc) as tc:
        with (
            tc.tile_pool(name="const", bufs=1) as cp,
            tc.tile_pool(name="outp", bufs=6) as outp,
            tc.tile_pool(name="maskp", bufs=8) as maskp,
            tc.tile_pool(name="ps", bufs=4, space="PSUM") as psp,
        ):
            # ---- setup: load inputs ----
            # Everything shares the sync HWDGE queue (using the scalar HWDGE
            # queue poisons DMA engine 15's throughput ~18% — measured), so
            # the queue is a single FIFO: packet order == issue order.  Batch
            # 0-7 need only HEAD (packed ys0+selectors, 128 KiB) and
            # idx[:, 0:32]; issue those first so batch 0's compute starts
            # ~2.5us after the barrier, then the bulk.
            IDX = cp.tile([S, BL * H], mybir.dt.int16, tag="IDX")
            LALL = cp.tile([96, 8 * 128], bf16, tag="LALL")
            YS = [cp.tile([S, V], bf16, name=f"ys{t}", tag=f"ys{t}") for t in range(3)]
            # HEAD packs batch 0-7's ys rows (cols 0:V) and selector columns
            # (cols V:2V) side by side: ONE DMA -> one completion semaphore
            # gates the first matmul (two serial small DMAs cost ~2us of
            # issue->completion latency each).
            HEAD = cp.tile([32, 2 * V], bf16, tag="HEAD")
            nc.sync.dma_start(out=HEAD[:], in_=head_d[:])
            nc.sync.dma_start(out=IDX[:, 0:32], in_=idx_d[:, 0:32])
            nc.sync.dma_start(out=IDX[:, 32 : BL * H], in_=idx_d[:, 32 : BL * H])
            nc.sync.dma_start(out=YS[0][32:96, :], in_=yspl_d[32:96, :])
            nc.sync.dma_start(out=LALL[:], in_=lall_d[:])

            # ---- warmups (no data deps, run immediately) ----
            # Dummy scatter: forces Bacc's ModifyPoolConfig + the ~2.5us
            # gpsimd library IRAM load to the front, overlapping input DMAs.
            DUMIDX = cp.tile([S, 2], mybir.dt.int16, tag="DUMIDX")
            nc.gpsimd.memset(DUMIDX[:], -1)
            DUMSC = cp.tile([S, 2], mybir.dt.int16, tag="DUMSC")
            nc.gpsimd.local_scatter(
                out_ap=DUMSC[:],
                data_ap=DUMIDX[:],
                idxs_ap=DUMIDX[:],
                channels=S,
                num_elems=2,
                num_idxs=2,
            )
            # Dummy Abs: hoists the ACT table load ahead of the pipeline.
            DUMF = cp.tile([S, 2], f32, tag="DUMF")
            nc.vector.memset(DUMF[:], 0.0)
            DUMF2 = cp.tile([S, 2], f32, tag="DUMF2")
            nc.scalar.activation(
                out=DUMF2[:], in_=DUMF[:], func=mybir.ActivationFunctionType.Abs
            )

            # ---- matmul weights (host-built constant) ----
            # PE K-windows must start at partition 0/32/64, so contract over
            # a full 32-partition window (8 batches) and use a selector lhsT
            # that zeroes every batch except slot r: column block r of LALL
            # has 1.0 in rows 4r..4r+2 (the bf16 hi/mid/lo splits of batch
            # r).  Pattern replicated at all three window bases so lhsT and
            # rhs slices share a base partition.

            # Scatter payload + wait-absorbers: InstISA (local_scatter) only
            # supports a limited number of semaphore waits, so satisfy its
            # cross-engine deps (IDX DMA, ONES init) on the gpsimd engine
            # itself; program order then covers them for every scatter.
            ONES = cp.tile([S, H], mybir.dt.int16, tag="ONES")  # scatter payload
            nc.gpsimd.memset(ONES[:], 1)
            IDXPROBE = cp.tile([S, 2], mybir.dt.int16, tag="IDXPROBE")
            nc.gpsimd.tensor_copy(out=IDXPROBE[:], in_=IDX[:, 0:2])
            IDXPROBE2 = cp.tile([S, 2], mybir.dt.int16, tag="IDXPROBE2")

            # ---- per-batch pipeline ----
            # out[s, v] = |ybc[s, v] - mask[s, v]|.  With mask in {0, 1} and
            # y in [0, 1) this equals y (no hit) or 1-y (hit), so no 1-y
            # broadcast is needed.
            #
            # DMA granularity ramps up so the output stream starts as early
            # as possible (the whole kernel is one saturated write stream;
            # exec time = first-packet time + bytes/BW): batch 0 ships in two
            # half-row DMAs as soon as each half clears the Abs, batches 1-7
            # ship per batch, and the steady state ships two batches per DMA
            # to halve per-op fixed costs and semaphore/DMA-issue traffic.
            def alloc_py():
                return psp.tile([S, V], f32, name="py")

            def alloc_ot():
                # Single allocation site with one shape: the pools size a
                # slot per (site, shape), so mixed shapes/sites balloon the
                # SBUF footprint and relocate everything else (measured ~20%
                # slowdown on all compute engines from the layout shift).
                return outp.tile([S, 2 * V], f32, name="ot")

            def alloc_mk():
                return maskp.tile([S, V], mybir.dt.int16, name="mk")

            def mm_scatter(b):
                ys = YS[b // 24]
                m = b % 24
                w, r = m // 8, m % 8
                base = 32 * w
                py = alloc_py()
                for h2 in range(2):
                    sl = slice(h2 * 512, (h2 + 1) * 512)
                    if b < 8:
                        lhsT = HEAD[0:32, V + r * 128 : V + (r + 1) * 128]
                        rhs = HEAD[0:32, sl]
                    else:
                        lhsT = LALL[base : base + 32, r * 128 : (r + 1) * 128]
                        rhs = ys[base : base + 32, sl]
                    nc.tensor.matmul(
                        out=py[:, sl],
                        lhsT=lhsT,
                        rhs=rhs,
                        start=True,
                        stop=True,
                    )
                mk = alloc_mk()
                nc.gpsimd.local_scatter(
                    out_ap=mk[:],
                    data_ap=ONES[:],
                    idxs_ap=IDX[:, H * b : H * b + H],
                    channels=S,
                    num_elems=V,
                    num_idxs=H,
                )
                return py, mk

            # batch 0: column-halved sub/abs/DMA (earliest first packet)
            py, mk = mm_scatter(0)
            ot0 = alloc_ot()
            for h2 in range(2):
                sl = slice(h2 * 512, (h2 + 1) * 512)
                nc.vector.tensor_tensor(
                    out=py[:, sl], in0=py[:, sl], in1=mk[:, sl], op=Op.subtract
                )
                nc.scalar.activation(
                    out=ot0[:, sl],
                    in_=py[:, sl],
                    func=mybir.ActivationFunctionType.Abs,
                )
                nc.sync.dma_start(out=out_d[0:S, sl], in_=ot0[:, sl])

            nc.sync.dma_start(out=YS[1][0:96, :], in_=yspl_d[96:192, :])
            nc.sync.dma_start(out=YS[2][0:64, :], in_=yspl_d[192:256, :])

            # batches 1-7: one DMA per batch
            for b in range(1, 8):
                py, mk = mm_scatter(b)
                ot = alloc_ot()
                nc.vector.tensor_tensor(out=py[:], in0=py[:], in1=mk[:], op=Op.subtract)
                nc.scalar.activation(
                    out=ot[:, 0:V], in_=py[:], func=mybir.ActivationFunctionType.Abs
                )
                nc.sync.dma_start(out=out_d[b * S : (b + 1) * S, :], in_=ot[:, 0:V])

            # batches 8-63: two batches per DMA
            nc.gpsimd.tensor_copy(out=IDXPROBE2[:], in_=IDX[:, 32:34])
            for p in range(4, BL // 2):
                ot = alloc_ot()
                for bi in range(2):
                    b = 2 * p + bi
                    py, mk = mm_scatter(b)
                    nc.vector.tensor_tensor(
                        out=py[:], in0=py[:], in1=mk[:], op=Op.subtract
                    )
                    nc.scalar.activation(
                        out=ot[:, bi * V : (bi + 1) * V],
                        in_=py[:],
                        func=mybir.ActivationFunctionType.Abs,
                    )
                nc.sync.dma_start(
                    out=out_d[2 * p * S : (2 * p + 2) * S, :].rearrange(
                        "(bi s) v -> s bi v", bi=2
                    ),
                    in_=ot[:].rearrange("s (bi v) -> s bi v", bi=2),
                )
    # Bacc.compile(): register alloc, event-sem generation (splits waits
    # beyond the ISA limit), library load insertion for local_scatter, and
    # extended-inst ISA codegen.
    nc.compile()
    return nc


def _get_nc():
    global _nc_cache
    if _nc_cache is None:
        _nc_cache = _build_bass()
    return _nc_cache


def _make_lall():
    import ml_dtypes

    pat = np.zeros((32, 8, 128), np.float32)
    for r in range(8):
        pat[4 * r : 4 * r + 3, r, :] = 1.0
    blk = pat.reshape(32, 8 * 128)
    return np.ascontiguousarray(
        np.concatenate([blk, blk, blk], axis=0).astype(ml_dtypes.bfloat16)
    )


def _prep_inputs(y, idx):
    """Slice the full inputs into per-core in_maps (host-side index massaging
    only: dtype narrowing, layout transpose, duplicate-hop sentinel)."""
    y = np.asarray(y, dtype=np.float32)
    ii = np.asarray(idx)
    i16 = ii.astype(np.int16)  # values in [0, 1024)
    # reference uses .set semantics: mark duplicate hops within a row so the
    # scatter writes each position once; local_scatter ignores negatives.
    dup = np.zeros(ii.shape, dtype=bool)
    for j in range(1, H):
        for k in range(j):
            dup[..., j] |= ii[..., j] == ii[..., k]
    i16[dup] = -1
    lall = _make_lall()
    import ml_dtypes

    bf = ml_dtypes.bfloat16
    hi = y.astype(bf)
    r1 = y - hi.astype(np.float32)
    mid = r1.astype(bf)
    lo = (r1 - mid.astype(np.float32)).astype(bf)  # exact: <=8 bits remain
    ones = np.ones_like(hi)
    yspl = np.stack([hi, mid, lo, ones], axis=1)  # (B, 4, V)
    in_maps = []
    orders = []
    for c in range(NCORES):
        sl = slice(c * BL, (c + 1) * BL)
        # Rotate each core's batch processing order so the 8 cores don't
        # write the same relative DRAM offsets in lockstep (decorrelates
        # cross-core DMA phase alignment; the straggler-engine draws hit
        # one engine with correlated traffic).  The NEFF is unchanged —
        # only the per-core input layout rotates; _run un-permutes rows.
        order = np.arange(BL)
        order = (order + 8 * c) % BL
        orders.append(order)
        yspl_c = yspl[sl][order].reshape(4 * BL, V)
        in_maps.append(
            {
                "yspl": np.ascontiguousarray(yspl_c),
                "idx16": np.ascontiguousarray(
                    i16[sl][order].transpose(1, 0, 2).reshape(S, BL * H)
                ),
                "lall": lall,
                "head": np.ascontiguousarray(
                    np.concatenate([yspl_c[0:32], lall[0:32]], axis=1)
                ),
            }
        )
    return in_maps, orders


def _run(y, idx, **spmd_kwargs):
    nc = _get_nc()
    in_maps, orders = _prep_inputs(y, idx)
    res = run_bass_kernel_spmd(nc, in_maps, core_ids=list(range(NCORES)), **spmd_kwargs)
    out = np.empty((B, S, V), dtype=np.float32)
    for c in range(NCORES):
        # loop position p computed batch orders[c][p]
        out[c * BL + orders[c]] = res.results[c]["out"].reshape(BL, S, V)
    return out, res


def kernel(a=None, b=None, c=None, y=None, idx=None, **_unused):
    # a, b, c are unused by the reference computation.
    out, _ = _run(y, idx)
    return out

